# revision 8
# baseline (speedup 1.0000x reference)
"""Trainium2 Bass kernel for BlockAxialDown (maxpool + axial attention + 1x1 conv + batchnorm).

Contract: kernel(**inputs) takes FULL unsharded inputs, returns FULL output.
Sharding: data-parallel over batch B=8 across 8 NeuronCores (1 image/core);
BatchNorm batch stats combined with a tiny (128,4) AllReduce; weights replicated.
"""

import sys

import numpy as np

for _p in ("/opt/trn_rl_repo", "/root/.axon_site/_ro/trn_rl_repo"):
    if _p not in sys.path:
        sys.path.append(_p)

B, C, H, W = 8, 128, 256, 256
H2, W2 = 128, 128
E = 2 * C
NPOS = H2 * W2
NCORES = 8
BN_EPS = 1e-5
DH = C // 2
SCALE = DH ** -0.5

_CACHE = {}


def _build_program():
    import concourse.tile as tile
    from concourse import bacc, mybir
    from concourse.alu_op_type import AluOpType
    from contextlib import ExitStack

    F32 = mybir.dt.float32
    AF = mybir.ActivationFunctionType
    P = 128

    nc = bacc.Bacc("TRN2", target_bir_lowering=False, debug=False, num_devices=NCORES)

    # ---- DRAM I/O ----
    x_d = nc.dram_tensor("x", [C, H, W], F32, kind="ExternalInput").ap()
    wq_w_d = nc.dram_tensor("wq_w", [C, C], F32, kind="ExternalInput").ap()
    wk_w_d = nc.dram_tensor("wk_w", [C, C], F32, kind="ExternalInput").ap()
    wv_w_d = nc.dram_tensor("wv_w", [C, C], F32, kind="ExternalInput").ap()
    wo_w_d = nc.dram_tensor("wo_w", [C, C], F32, kind="ExternalInput").ap()
    wq_h_d = nc.dram_tensor("wq_h", [C, C], F32, kind="ExternalInput").ap()
    wk_h_d = nc.dram_tensor("wk_h", [C, C], F32, kind="ExternalInput").ap()
    wv_h_d = nc.dram_tensor("wv_h", [C, C], F32, kind="ExternalInput").ap()
    wo_h_d = nc.dram_tensor("wo_h", [C, C], F32, kind="ExternalInput").ap()
    bsum_d = nc.dram_tensor("bsum", [C, 1], F32, kind="ExternalInput").ap()
    convA_d = nc.dram_tensor("convA", [C, E], F32, kind="ExternalInput").ap()
    convX_d = nc.dram_tensor("convX", [C, E], F32, kind="ExternalInput").ap()
    gamma2_d = nc.dram_tensor("gamma2", [C, 2], F32, kind="ExternalInput").ap()
    beta2_d = nc.dram_tensor("beta2", [C, 2], F32, kind="ExternalInput").ap()
    out_d = nc.dram_tensor("out", [E, H2, W2], F32, kind="ExternalOutput").ap()
    stats_in_d = nc.dram_tensor("stats_in", [P, 4], F32).ap()
    stats_out_d = nc.dram_tensor("stats_out", [P, 4], F32, addr_space="Shared").ap()

    with tile.TileContext(nc) as tc, ExitStack() as ctx:
        const = ctx.enter_context(tc.tile_pool(name="const", bufs=1))
        cube = ctx.enter_context(tc.tile_pool(name="cube", bufs=1))
        stage = ctx.enter_context(tc.tile_pool(name="stage", bufs=3))
        work = ctx.enter_context(tc.tile_pool(name="work", bufs=2))
        stats = ctx.enter_context(tc.tile_pool(name="stats", bufs=1))
        psum = ctx.enter_context(tc.tile_pool(name="psum", bufs=1, space="PSUM"))

        # ---- constants ----
        def cload(name, ap_d, shape):
            t = const.tile(shape, F32, name=name)
            nc.sync.dma_start(out=t[:], in_=ap_d)
            return t

        wq_w = cload("wq_w_t", wq_w_d, [C, C])
        wk_w = cload("wk_w_t", wk_w_d, [C, C])
        wv_w = cload("wv_w_t", wv_w_d, [C, C])
        wo_w = cload("wo_w_t", wo_w_d, [C, C])
        wq_h = cload("wq_h_t", wq_h_d, [C, C])
        wk_h = cload("wk_h_t", wk_h_d, [C, C])
        wv_h = cload("wv_h_t", wv_h_d, [C, C])
        wo_h = cload("wo_h_t", wo_h_d, [C, C])
        bsum = cload("bsum_t", bsum_d, [C, 1])
        convA = cload("convA_t", convA_d, [C, E])
        convX = cload("convX_t", convX_d, [C, E])
        gamma2 = cload("gamma2_t", gamma2_d, [C, 2])
        beta2 = cload("beta2_t", beta2_d, [C, 2])

        ones_col = const.tile([P, 1], F32)
        nc.vector.memset(ones_col[:], 1.0)
        ones_row = const.tile([1, P], F32)
        nc.vector.memset(ones_row[:], 1.0)

        xp = cube.tile([P, H2, W2], F32)   # pooled input, channels on partitions
        acc = cube.tile([P, H2, W2], F32)  # attention output accumulator
        xp_f = xp[:].rearrange("c h w -> c (h w)")
        acc_f = acc[:].rearrange("c h w -> c (h w)")

        # ---- phase 1: load + 2x2 maxpool ----
        xv = x_d.rearrange("c (n r) w -> c n r w", r=8)
        for i in range(H // 8):
            xin = stage.tile([P, 8, W], F32, tag="xin")
            nc.sync.dma_start(out=xin[:], in_=xv[:, i])
            t = stage.tile([P, 8, W2], F32, tag="wmax")
            xin4 = xin[:].rearrange("c r (w two) -> c r w two", two=2)
            nc.vector.tensor_max(t[:], xin4[:, :, :, 0], xin4[:, :, :, 1])
            t4 = t[:].rearrange("c (r2 two) w -> c r2 two w", two=2)
            nc.vector.tensor_max(xp[:, 4 * i:4 * i + 4, :], t4[:, :, 0, :], t4[:, :, 1, :])

        # ---- axial attention over a group of 4 slices ----
        # rhs_g: (c, 4, t) AP, slice-major. Returns yT psum tile (c_out, 4*t).
        def attn_group(rhs_g, wq, wk, wv, wo):
            qg_ps = psum.tile([P, 512], F32, tag="proj_q")
            nc.tensor.matmul(qg_ps[:], lhsT=wq[:], rhs=rhs_g, start=True, stop=True)
            kg_ps = psum.tile([P, 512], F32, tag="proj_k")
            nc.tensor.matmul(kg_ps[:], lhsT=wk[:], rhs=rhs_g, start=True, stop=True)
            qg = work.tile([P, 512], F32, tag="qg")
            nc.scalar.copy(qg[:], qg_ps[:])
            kg = work.tile([P, 512], F32, tag="kg")
            nc.vector.tensor_copy(kg[:], kg_ps[:])
            og = work.tile([P, 512], F32, tag="og")
            for s in range(4):
                cs = slice(128 * s, 128 * s + 128)
                xs = rhs_g[:, s, :]
                v_ps = psum.tile([P, 128], F32, tag="vr", name="v_ps")
                nc.tensor.matmul(v_ps[:], lhsT=xs, rhs=wv[:], start=True, stop=True)
                vs = work.tile([P, 128], F32, tag="vs")
                nc.scalar.copy(vs[:], v_ps[:])
                dots0 = psum.tile([P, 128], F32, tag="dots0")
                nc.tensor.matmul(dots0[:], lhsT=kg[0:64, cs], rhs=qg[0:64, cs],
                                 start=True, stop=True)
                dots1 = psum.tile([P, 128], F32, tag="dots1")
                nc.tensor.matmul(dots1[:], lhsT=kg[64:128, cs], rhs=qg[64:128, cs],
                                 start=True, stop=True)
                e = work.tile([P, 256], F32, tag="e")
                nc.scalar.activation(e[:, 0:128], dots0[:], AF.Exp, scale=SCALE)
                nc.scalar.activation(e[:, 128:256], dots1[:], AF.Exp, scale=SCALE)
                rsum = psum.tile([1, 256], F32, tag="vr", name="rsum")
                nc.tensor.matmul(rsum[0:1, :], lhsT=ones_col[:], rhs=e[:],
                                 start=True, stop=True)
                rs = work.tile([1, 256], F32, tag="rs")
                nc.vector.tensor_copy(rs[:], rsum[0:1, :])
                rb0 = psum.tile([P, 128], F32, tag="g0", name="rb0")
                nc.tensor.matmul(rb0[0:64, :], lhsT=ones_row[0:1, 0:64], rhs=rs[0:1, 0:128],
                                 start=True, stop=True)
                rb1 = psum.tile([P, 128], F32, tag="g1", name="rb1")
                nc.tensor.matmul(rb1[64:128, :], lhsT=ones_row[0:1, 0:64], rhs=rs[0:1, 128:256],
                                 start=True, stop=True, tile_position=(0, 64))
                rbs = work.tile([P, 128], F32, tag="rbs")
                nc.vector.reciprocal(rbs[0:64, :], rb0[0:64, :])
                nc.vector.reciprocal(rbs[64:128, :], rb1[64:128, :])
                oT0 = psum.tile([P, 128], F32, tag="g0", name="oT0")
                nc.tensor.matmul(oT0[0:64, :], lhsT=vs[:, 0:64], rhs=e[:, 0:128],
                                 start=True, stop=True)
                oT1 = psum.tile([P, 128], F32, tag="g1", name="oT1")
                nc.tensor.matmul(oT1[64:128, :], lhsT=vs[:, 64:128], rhs=e[:, 128:256],
                                 start=True, stop=True, tile_position=(0, 64))
                nc.vector.tensor_mul(og[0:64, cs], oT0[0:64, :], rbs[0:64, :])
                nc.vector.tensor_mul(og[64:128, cs], oT1[64:128, :], rbs[64:128, :])
            yg_ps = psum.tile([P, 512], F32, tag="yT")
            nc.tensor.matmul(yg_ps[:], lhsT=wo[:], rhs=og[:], start=True, stop=True)
            return yg_ps

        # ---- phase 2: W-direction attention (rows contiguous) ----
        for g in range(H2 // 4):
            rhs_g = xp[:, 4 * g:4 * g + 4, :]
            yg = attn_group(rhs_g, wq_w, wk_w, wv_w, wo_w)
            # acc = yT_w + (bout_h + bout_w), contiguous write
            nc.scalar.activation(acc_f[:, 512 * g:512 * (g + 1)], yg[:],
                                 AF.Identity, bias=bsum[:, 0:1], scale=1.0)

        # ---- phase 3: H-direction attention (columns, strided) ----
        for g in range(W2 // 4):
            rhs_g = xp[:, :, 4 * g:4 * g + 4].rearrange("c h w -> c w h")
            yg = attn_group(rhs_g, wq_h, wk_h, wv_h, wo_h)
            # accumulate transposed: acc[:, h, w] += yg[:, (s=w, i=h)]
            acc_sl = acc[:, :, 4 * g:4 * g + 4]
            yg_r = yg[:].rearrange("c (s i) -> c i s", s=4)
            nc.vector.tensor_add(acc_sl, acc_sl, yg_r)

        # ---- phase 3.5: relu over acc ----
        for j in range(4):
            sl = acc_f[:, 4096 * j:4096 * (j + 1)]
            nc.vector.tensor_scalar_max(sl, sl, 0.0)

        # ---- conv + relu for one (chunk, out-half) ----
        def conv_chunk(p, eh):
            yps = psum.tile([P, 512], F32, tag="proj_q" if eh == 0 else "proj_k")
            ce = slice(128 * eh, 128 * eh + 128)
            pos = slice(512 * p, 512 * (p + 1))
            nc.tensor.matmul(yps[:], lhsT=convA[:, ce], rhs=acc_f[:, pos],
                             start=True, stop=False)
            nc.tensor.matmul(yps[:], lhsT=convX[:, ce], rhs=xp_f[:, pos],
                             start=False, stop=True)
            yr = work.tile([P, 512], F32, tag=f"yr{eh}")
            nc.scalar.activation(yr[:], yps[:], AF.Relu)
            return yr

        # ---- phase 4: conv pass 1, stats only ----
        bnb = [stats.tile([P, 32, 6], F32, name=f"bnb{i}") for i in range(2)]
        for p in range(NPOS // 512):
            for eh in range(2):
                yr = conv_chunk(p, eh)
                nc.vector.bn_stats(bnb[eh][:, p, :], yr[:])

        mv = stats.tile([P, 2, 2], F32)
        for eh in range(2):
            nc.vector.bn_aggr(mv[:, eh, :], bnb[eh][:])
        cc_in = stats.tile([P, 4], F32)
        for eh in range(2):
            # [mean, E[y^2]] per half; E[y^2] = var + mean^2
            nc.vector.tensor_copy(cc_in[:, 2 * eh:2 * eh + 1], mv[:, eh, 0:1])
            nc.vector.scalar_tensor_tensor(
                cc_in[:, 2 * eh + 1:2 * eh + 2],
                in0=mv[:, eh, 0:1], scalar=mv[:, eh, 0:1], in1=mv[:, eh, 1:2],
                op0=AluOpType.mult, op1=AluOpType.add)
        nc.sync.dma_start(out=stats_in_d, in_=cc_in[:])
        nc.gpsimd.collective_compute(
            "AllReduce", AluOpType.add,
            replica_groups=[list(range(NCORES))],
            ins=[stats_in_d], outs=[stats_out_d])
        gst = stats.tile([P, 4], F32)
        nc.sync.dma_start(out=gst[:], in_=stats_out_d)

        # ---- phase 5: BN affine coefficients ----
        t0 = stats.tile([P, 4], F32)
        nc.vector.tensor_scalar_mul(t0[:], gst[:], 1.0 / NCORES)
        t0v = t0[:].rearrange("c (e two) -> c e two", two=2)
        m2 = stats.tile([P, 2], F32)
        veps = stats.tile([P, 2], F32)
        for eh in range(2):
            nc.vector.tensor_mul(m2[:, eh:eh + 1], t0v[:, eh, 0:1], t0v[:, eh, 0:1])
            nc.vector.scalar_tensor_tensor(
                veps[:, eh:eh + 1],
                in0=t0v[:, eh, 1:2], scalar=BN_EPS, in1=m2[:, eh:eh + 1],
                op0=AluOpType.add, op1=AluOpType.subtract)
        sd = stats.tile([P, 2], F32)
        nc.scalar.sqrt(sd[:], veps[:])
        rstd = stats.tile([P, 2], F32)
        nc.vector.reciprocal(rstd[:], sd[:])
        scl = stats.tile([P, 2], F32)
        nc.vector.tensor_mul(scl[:], gamma2[:], rstd[:])
        msc = stats.tile([P, 2], F32)
        means = stats.tile([P, 2], F32)
        nc.vector.tensor_copy(means[:, 0:1], t0v[:, 0, 0:1])
        nc.vector.tensor_copy(means[:, 1:2], t0v[:, 1, 0:1])
        nc.vector.tensor_mul(msc[:], means[:], scl[:])
        shift = stats.tile([P, 2], F32)
        nc.vector.tensor_sub(shift[:], beta2[:], msc[:])

        # ---- phase 6: conv pass 2, affine, output ----
        out_r = out_d.rearrange("(two c) h w -> two c (h w)", two=2)
        for p in range(NPOS // 512):
            for eh in range(2):
                yr = conv_chunk(p, eh)
                yo = work.tile([P, 512], F32, tag=f"yo{eh}")
                nc.vector.tensor_scalar(
                    yo[:], yr[:], scl[:, eh:eh + 1], shift[:, eh:eh + 1],
                    op0=AluOpType.mult, op1=AluOpType.add)
                nc.sync.dma_start(out=out_r[eh, :, 512 * p:512 * (p + 1)], in_=yo[:])

    nc.finalize()
    return nc


def _get_program():
    if "nc" not in _CACHE:
        _CACHE["nc"] = _build_program()
    return _CACHE["nc"]


def _make_in_maps(x, Wq_h, Wkv_h, Wout_h, bout_h, Wq_w, Wkv_w, Wout_w, bout_w,
                  conv_w, gamma, beta):
    f = np.float32
    shared = {
        "wq_w": np.ascontiguousarray(Wq_w, f),
        "wk_w": np.ascontiguousarray(Wkv_w[:, :C], f),
        "wv_w": np.ascontiguousarray(Wkv_w[:, C:], f),
        "wo_w": np.ascontiguousarray(Wout_w, f),
        "wq_h": np.ascontiguousarray(Wq_h, f),
        "wk_h": np.ascontiguousarray(Wkv_h[:, :C], f),
        "wv_h": np.ascontiguousarray(Wkv_h[:, C:], f),
        "wo_h": np.ascontiguousarray(Wout_h, f),
        "bsum": np.ascontiguousarray((np.asarray(bout_h) + np.asarray(bout_w)).reshape(C, 1), f),
        "convA": np.ascontiguousarray(conv_w[:C, :], f),
        "convX": np.ascontiguousarray(conv_w[C:, :], f),
        "gamma2": np.ascontiguousarray(np.asarray(gamma).reshape(2, C).T, f),
        "beta2": np.ascontiguousarray(np.asarray(beta).reshape(2, C).T, f),
    }
    x = np.asarray(x, f)
    return [{**shared, "x": np.ascontiguousarray(x[b])} for b in range(B)]


def run(trace=False, **inputs):
    from concourse.bass_utils import run_bass_kernel_spmd

    nc = _get_program()
    in_maps = _make_in_maps(**inputs)
    res = run_bass_kernel_spmd(nc, in_maps, list(range(NCORES)), trace=trace)
    out = np.stack([res.results[b]["out"] for b in range(B)], axis=0)
    return out, res


def kernel(**inputs):
    out, _ = run(trace=False, **inputs)
    return out
